# revision 31
# baseline (speedup 1.0000x reference)
"""Instant-NGP hash-encoding forward on 8 TRN2 NeuronCores.

Data-parallel over points (8 cores). Per core:
  - fp16 per-level tables (padded to 16384 entries) broadcast-DMA'd into SBUF
    across all 128 partitions (64KB/partition), one level at a time.
  - Gather via GPSIMD ap_gather (8 Q7 cores/NC work in parallel on their own
    16-partition groups).
  - DVE computes cell coords (floor with round-to-nearest fix), spatial-hash
    indices (int32 mul/and/xor; mod 2^14 == low-14-bit arithmetic), trilinear
    weights, and the 8-corner weighted reduction.
  - Per-level results go to a DRAM scratch (level-major); a final pass
    interleaves them into the [N, 32] output (fp16 to halve the fetch).

Host path: one cached AOT-compiled jax executable (fresh jit per call would
retrace + recompile); output zero-buffers are created on device instead of
being shipped; output returns as fp16 and is upcast host-side. All of this
matters because the axon tunnel runs at ~45MB/s each way.
"""
from concurrent.futures import ThreadPoolExecutor

import numpy as np

import jax
import jax.numpy as jnp
from jax.sharding import (Mesh, PartitionSpec, NamedSharding,
                          SingleDeviceSharding)
from jax.experimental.shard_map import shard_map

import concourse.bass as bass
import concourse.mybir as mybir
from concourse import bacc, bass2jax
from concourse.tile import TileContext
from concourse.bass_utils import run_bass_kernel_spmd

F32 = mybir.dt.float32
F16 = mybir.dt.float16
I32 = mybir.dt.int32
I16 = mybir.dt.int16
I8 = mybir.dt.int8
U16 = mybir.dt.uint16
AL = mybir.AluOpType
AX = mybir.AxisListType

NUM_LEVELS = 16
TABLE_SIZE = 2 ** 14
MIN_RES, MAX_RES = 16, 512
FEAT = 2
N_POINTS = 1 << 20
N_CORES = 8
PI1, PI2 = 2654435761, 805459861
P1L = PI1 & (TABLE_SIZE - 1)
P2L = PI2 & (TABLE_SIZE - 1)

_b = np.exp((np.log(MAX_RES) - np.log(MIN_RES)) / (NUM_LEVELS - 1))
RES = np.floor(MIN_RES * _b ** np.arange(NUM_LEVELS)).astype(np.int64)
COUNTS = np.minimum((RES + 1) ** 3, TABLE_SIZE)
OFFSETS = np.concatenate([[0], np.cumsum(COUNTS)])
DENSE = [int(COUNTS[l]) == int((RES[l] + 1) ** 3) for l in range(NUM_LEVELS)]

N_SPLIT = 4                      # sequential chunk-calls: exec(i+1) overlaps fetch(i)
NH = N_POINTS // N_SPLIT
NC_N = NH // N_CORES             # 65536 points per core per call
P = 128
PPP = NC_N // P                  # 512 points per partition
T = 64                           # points per partition per tile
NT = PPP // T                    # tiles per core
NI = 16 * T * 8                  # ap_gather num_idxs per 16-partition group
NE = TABLE_SIZE

# f32 blob slots (units of T elements)
S_SX, S_XF, S_GT = 0, 1, 2
S_FL = 3   # 3 slots
S_FR = 6   # 3 slots
S_W0 = 9   # 3 slots
S_WXY = 12  # 4 slots
S_PROD = 16  # 16 slots
NBF = 32
# i32 blob slots
S_XI = 0
S_FI = 1   # 3 slots
S_HX1 = 4
S_HY0, S_HY1, S_HZ0, S_HZ1 = 5, 6, 7, 8
S_TMP = 9
S_HXY = 10  # 4 slots
S_IDX = 14  # 8 slots
NBI = 22


def _ap(tile_ap, part_off, part_step, part_cnt, elem_off, dims):
    pitch = tile_ap.ap[0][0]
    return bass.AP(
        tile_ap.tensor,
        tile_ap.offset + part_off * pitch + elem_off,
        [[part_step * pitch, part_cnt]] + dims,
    )


def _build_nc():
    nc = bacc.Bacc("TRN2", target_bir_lowering=False, debug=False)
    coords = nc.dram_tensor("coords", [NC_N, 3], U16, kind="ExternalInput")
    emb16 = nc.dram_tensor("emb16", [NUM_LEVELS, NE * FEAT], F16, kind="ExternalInput")
    # int8 quantized output + per-point fp16 scale: out = q * s / 127.
    outq = nc.dram_tensor("outq", [NC_N, 32], I8, kind="ExternalOutput")
    outs = nc.dram_tensor("outs", [NC_N], F16, kind="ExternalOutput")

    coords_v = coords[:, :].rearrange("(p q) c -> p (q c)", p=P)
    outq_v = outq[:, :].rearrange("(p q) g -> p (q g)", p=P)
    outs_v = outs[:].rearrange("(p q) -> p q", p=P)

    with TileContext(nc) as tc:
        with tc.tile_pool(name="dram", bufs=1, space="DRAM") as dpool, \
             tc.tile_pool(name="tab", bufs=1) as tabp, \
             tc.tile_pool(name="coord", bufs=1) as cpool, \
             tc.tile_pool(name="gat", bufs=2) as gpool, \
             tc.tile_pool(name="blob", bufs=2) as bp, \
             tc.tile_pool(name="idxw", bufs=2) as xp, \
             tc.tile_pool(name="io", bufs=2) as iop:

            scratch = dpool.tile([NUM_LEVELS * NT * P, T * FEAT], F32)

            ct = cpool.tile([P, PPP * 3], U16)
            nc.sync.dma_start(out=ct[:], in_=coords_v)

            for l in range(NUM_LEVELS):
                R = int(RES[l])
                tab = tabp.tile([P, NE * FEAT], F16, tag="tab")
                nc.sync.dma_start(
                    out=tab[:], in_=emb16[l:l + 1, :].to_broadcast([P, NE * FEAT])
                )
                for ti in range(NT):
                    co = ti * T * 3
                    cap = ct[:]
                    cviews = [
                        bass.AP(cap.tensor, cap.offset + co + a, [cap.ap[0], [3, T]])
                        for a in range(3)
                    ]

                    bf = bp.tile([P, NBF * T], F32, tag="bf")
                    bi = bp.tile([P, NBI * T], I32, tag="bi")

                    def fv(slot, dims=None, off=0):
                        return _ap(bf[:], 0, 1, P, slot * T + off, dims or [[1, T]])

                    def iv(slot, dims=None, off=0):
                        return _ap(bi[:], 0, 1, P, slot * T + off, dims or [[1, T]])

                    # floor + frac per axis (coords are u16 fixed-point / 2^16)
                    for a in range(3):
                        nc.vector.tensor_copy(out=fv(S_GT), in_=cviews[a])
                        nc.vector.tensor_scalar(out=fv(S_SX), in0=fv(S_GT),
                                                scalar1=float(R) / 65536.0,
                                                scalar2=None, op0=AL.mult)
                        nc.vector.tensor_copy(out=iv(S_XI), in_=fv(S_SX))
                        nc.vector.tensor_copy(out=fv(S_XF), in_=iv(S_XI))
                        nc.vector.tensor_tensor(out=fv(S_GT), in0=fv(S_XF), in1=fv(S_SX),
                                                op=AL.is_gt)
                        nc.vector.tensor_tensor(out=fv(S_FL + a), in0=fv(S_XF),
                                                in1=fv(S_GT), op=AL.subtract)
                        nc.vector.tensor_tensor(out=fv(S_FR + a), in0=fv(S_SX),
                                                in1=fv(S_FL + a), op=AL.subtract)
                        nc.vector.tensor_copy(out=iv(S_FI + a), in_=fv(S_FL + a))

                    if DENSE[l]:
                        Rp = R + 1
                        nc.vector.tensor_scalar(out=iv(S_HX1), in0=iv(S_FI + 1),
                                                scalar1=Rp, scalar2=None, op0=AL.mult)
                        nc.vector.tensor_tensor(out=iv(S_HY0), in0=iv(S_HX1),
                                                in1=iv(S_FI + 0), op=AL.add)
                        nc.vector.tensor_scalar(out=iv(S_HY1), in0=iv(S_FI + 2),
                                                scalar1=Rp * Rp, scalar2=None, op0=AL.mult)
                        nc.vector.tensor_tensor(out=iv(S_HZ0), in0=iv(S_HY0),
                                                in1=iv(S_HY1), op=AL.add)
                        for c in range(8):
                            i, j, k = (c >> 2) & 1, (c >> 1) & 1, c & 1
                            doff = i + Rp * j + Rp * Rp * k
                            ov = iv(S_IDX, [[8, T]], off=c)
                            nc.vector.tensor_scalar(out=ov, in0=iv(S_HZ0), scalar1=doff,
                                                    scalar2=None, op0=AL.add)
                    else:
                        nc.vector.tensor_scalar(out=iv(S_HX1), in0=iv(S_FI + 0),
                                                scalar1=1, scalar2=None, op0=AL.add)
                        for ax, pl, s0, s1 in ((1, P1L, S_HY0, S_HY1),
                                               (2, P2L, S_HZ0, S_HZ1)):
                            nc.vector.tensor_scalar(out=iv(S_TMP), in0=iv(S_FI + ax),
                                                    scalar1=pl, scalar2=None, op0=AL.mult)
                            nc.vector.tensor_scalar(out=iv(s0), in0=iv(S_TMP),
                                                    scalar1=NE - 1, scalar2=None,
                                                    op0=AL.bitwise_and)
                            nc.vector.tensor_scalar(out=iv(S_TMP), in0=iv(s0),
                                                    scalar1=pl, scalar2=None, op0=AL.add)
                            nc.vector.tensor_scalar(out=iv(s1), in0=iv(S_TMP),
                                                    scalar1=NE - 1, scalar2=None,
                                                    op0=AL.bitwise_and)
                        for i in range(2):
                            hxs = iv(S_FI + 0) if i == 0 else iv(S_HX1)
                            for j in range(2):
                                ov = iv(S_HXY, [[4, T]], off=i * 2 + j)
                                nc.vector.tensor_tensor(out=ov, in0=hxs,
                                                        in1=iv(S_HY0 if j == 0 else S_HY1),
                                                        op=AL.bitwise_xor)
                        for c in range(8):
                            i, j, k = (c >> 2) & 1, (c >> 1) & 1, c & 1
                            inv = iv(S_HXY, [[4, T]], off=i * 2 + j)
                            ov = iv(S_IDX, [[8, T]], off=c)
                            nc.vector.tensor_tensor(out=ov, in0=inv,
                                                    in1=iv(S_HZ0 if k == 0 else S_HZ1),
                                                    op=AL.bitwise_xor)

                    idx16 = xp.tile([P, T * 8], I16, tag="idx16")
                    nc.vector.tensor_copy(out=idx16[:],
                                          in_=iv(S_IDX, [[1, 8 * T]]))

                    # weights
                    for a in range(3):
                        nc.vector.tensor_scalar(out=fv(S_W0 + a), in0=fv(S_FR + a),
                                                scalar1=-1.0, scalar2=1.0,
                                                op0=AL.mult, op1=AL.add)
                    for i in range(2):
                        for j in range(2):
                            ov = fv(S_WXY, [[4, T]], off=i * 2 + j)
                            nc.vector.tensor_tensor(
                                out=ov, in0=fv(S_W0 + 0 if i == 0 else S_FR + 0),
                                in1=fv(S_W0 + 1 if j == 0 else S_FR + 1), op=AL.mult)
                    wt = xp.tile([P, T * 8], F32, tag="wt")
                    for c in range(8):
                        i, j, k = (c >> 2) & 1, (c >> 1) & 1, c & 1
                        inv = fv(S_WXY, [[4, T]], off=i * 2 + j)
                        ov = _ap(wt[:], 0, 1, P, c, [[8, T]])
                        nc.vector.tensor_tensor(out=ov, in0=inv,
                                                in1=fv(S_W0 + 2 if k == 0 else S_FR + 2),
                                                op=AL.mult)

                    gat = gpool.tile([P, NI * FEAT], F16, tag="gat")
                    nc.gpsimd.ap_gather(
                        out_ap=gat[:], in_ap=tab[:], idxs_ap=idx16[:],
                        channels=P, num_elems=NE, d=FEAT, num_idxs=NI,
                    )

                    # de-interleave: partition 16g+j's results live at slots
                    # s*16+j (replicated across the group); 16 partition-subset
                    # DMAs bring each partition its own (t,c,f)-ordered copy.
                    gx = xp.tile([P, T * 16], F16, tag="gx")
                    for j in range(16):
                        src = _ap(gat[:], j, 16, 8, j * 2, [[32, 8 * T], [1, 2]])
                        dst = _ap(gx[:], j, 16, 8, 0, [[1, 16 * T]])
                        nc.sync.dma_start(out=dst, in_=src)

                    res = iop.tile([P, T * FEAT], F32, tag="res")
                    gv = gx[:].rearrange("p (t c f) -> p t f c", c=8, f=2)
                    wv = _ap(wt[:], 0, 1, P, 0, [[8, T], [0, 2], [1, 8]])
                    pv = _ap(bf[:], 0, 1, P, S_PROD * T, [[16, T], [8, 2], [1, 8]])
                    nc.vector.tensor_tensor(out=pv, in0=gv, in1=wv, op=AL.mult)
                    pv2 = _ap(bf[:], 0, 1, P, S_PROD * T, [[16, T], [8, 2], [1, 8]])
                    rv = res[:].rearrange("p (t f) -> p t f", f=2)
                    nc.vector.tensor_reduce(out=rv, in_=pv2, axis=AX.X, op=AL.add)

                    row = (l * NT + ti) * P
                    nc.sync.dma_start(out=scratch[row:row + P, :], in_=res[:])

            for ti in range(NT):
                asm = iop.tile([P, T * 32], F16, tag="asm")
                for l in range(NUM_LEVELS):
                    slab = iop.tile([P, T * FEAT], F32, tag="slab")
                    row = (l * NT + ti) * P
                    nc.sync.dma_start(out=slab[:], in_=scratch[row:row + P, :])
                    av = asm[:].rearrange("p (t g) -> p t g", g=32)[:, :, 2 * l:2 * l + 2]
                    sv = slab[:].rearrange("p (t f) -> p t f", f=FEAT)
                    nc.vector.tensor_copy(out=av, in_=sv)

                # int8 quantize with per-point scale: s16 = f16(max|x|),
                # q = round(x * 127 / s16); host decodes q * (s16/127).
                babs = iop.tile([P, T * 32], F16, tag="babs")
                s16p = iop.tile([P, T], F16, tag="s16p")
                s16 = iop.tile([P, T], F16, tag="s16")
                rs = iop.tile([P, T], F32, tag="rs")
                rs2 = iop.tile([P, T], F32, tag="rs2")
                rs3 = iop.tile([P, T], F32, tag="rs3")
                q8 = iop.tile([P, T * 32], I8, tag="q8")
                av3 = asm[:].rearrange("p (t g) -> p t g", g=32)
                # |x| = clear the f16 sign bit
                nc.vector.tensor_scalar(out=babs[:].bitcast(I16),
                                        in0=asm[:].bitcast(I16),
                                        scalar1=0x7FFF, scalar2=None,
                                        op0=AL.bitwise_and)
                ab3 = babs[:].rearrange("p (t g) -> p t g", g=32)
                nc.vector.tensor_reduce(out=s16p[:], in_=ab3, axis=AX.X,
                                        op=AL.max)
                nc.vector.tensor_scalar(out=s16[:], in0=s16p[:], scalar1=1e-6,
                                        scalar2=None, op0=AL.max)
                nc.vector.tensor_copy(out=rs[:], in_=s16[:])
                nc.vector.reciprocal(out=rs2[:], in_=rs[:])
                nc.vector.tensor_scalar(out=rs3[:], in0=rs2[:], scalar1=127.0,
                                        scalar2=None, op0=AL.mult)
                q3 = q8[:].rearrange("p (t g) -> p t g", g=32)
                scb = _ap(rs3[:], 0, 1, P, 0, [[1, T], [0, 32]])
                nc.vector.tensor_tensor(out=q3, in0=av3, in1=scb, op=AL.mult)
                nc.sync.dma_start(
                    out=bass.AP(outq_v.tensor, outq_v.offset + ti * T * 32,
                                [outq_v.ap[0], [1, T * 32]]),
                    in_=q8[:],
                )
                nc.sync.dma_start(
                    out=bass.AP(outs_v.tensor, outs_v.offset + ti * T,
                                [outs_v.ap[0], [1, T]]),
                    in_=s16[:],
                )
    nc.compile()
    return nc


_NC_CACHE = None
_COMPILED_CACHE = None


def _get_nc():
    global _NC_CACHE
    if _NC_CACHE is None:
        _NC_CACHE = _build_nc()
    return _NC_CACHE


def _get_compiled():
    """One AOT-compiled 8-device executable, cached for the process.

    Mirrors run_bass_via_pjrt's lowering but (a) is compiled once instead of
    per call, (b) materializes the ExternalOutput zero-buffers on device
    (saves a 64MB host->device ship per call), (c) uses the replicated
    sharding for the embedding table instead of 8 concatenated copies.
    """
    global _COMPILED_CACHE
    if _COMPILED_CACHE is None:
        nc = _get_nc()
        bass2jax.install_neuronx_cc_hook()
        assert nc.dbg_addr is None
        partition_name = (
            nc.partition_id_tensor.name if nc.partition_id_tensor else None
        )

        in_names, out_names, out_avals = [], [], []
        for alloc in nc.m.functions[0].allocations:
            if not isinstance(alloc, mybir.MemoryLocationSet):
                continue
            name = alloc.memorylocations[0].name
            if alloc.kind == "ExternalInput":
                if name != partition_name:
                    in_names.append(name)
            elif alloc.kind == "ExternalOutput":
                out_names.append(name)
                out_avals.append(jax.core.ShapedArray(
                    tuple(alloc.tensor_shape), mybir.dt.np(alloc.dtype)))
        assert in_names == ["coords", "emb16"], in_names
        assert out_names == ["outq", "outs"], out_names
        bind_in_names = tuple(in_names) + tuple(out_names)
        if partition_name is not None:
            bind_in_names = bind_in_names + (partition_name,)

        devices = jax.devices()[:N_CORES]
        mesh = Mesh(np.asarray(devices), ("core",))

        def _body(coords_l, emb_l, zq_l, zs_l):
            operands = [coords_l, emb_l, zq_l, zs_l]
            if partition_name is not None:
                operands.append(bass2jax.partition_id_tensor())
            outs = bass2jax._bass_exec_p.bind(
                *operands,
                out_avals=tuple(out_avals),
                in_names=bind_in_names,
                out_names=tuple(out_names),
                lowering_input_output_aliases=(),
                sim_require_finite=True,
                sim_require_nnan=True,
                nc=nc,
            )
            return tuple(outs)

        fn = shard_map(
            _body, mesh=mesh,
            in_specs=(PartitionSpec("core"), PartitionSpec(),
                      PartitionSpec("core"), PartitionSpec("core")),
            out_specs=(PartitionSpec("core"), PartitionSpec("core")),
            check_rep=False,
        )
        sh_core = jax.sharding.NamedSharding(mesh, PartitionSpec("core"))
        # The output "zero buffer" parameters run_bass_via_pjrt ships from the
        # host every call; our kernel writes every output element, so
        # device-resident dummies (never donated, so they survive calls) work.
        zeros = jax.jit(
            lambda: (jnp.zeros((NH, 32), jnp.int8),
                     jnp.zeros((NH,), jnp.float16)),
            out_shardings=(sh_core, sh_core),
        )()
        jax.block_until_ready(zeros)
        cshape = jax.ShapeDtypeStruct((NH, 3), np.uint16)
        eshape = jax.ShapeDtypeStruct((NUM_LEVELS, NE * FEAT), np.float16)
        zqshape = jax.ShapeDtypeStruct((NH, 32), np.int8)
        zsshape = jax.ShapeDtypeStruct((NH,), np.float16)
        compiled = bass2jax.fast_dispatch_compile(
            lambda: jax.jit(fn).lower(cshape, eshape, zqshape, zsshape).compile()
        )
        shardings = {
            "core": sh_core,
            "rep": jax.sharding.NamedSharding(mesh, PartitionSpec()),
            "dev0": SingleDeviceSharding(devices[0]),
        }
        _COMPILED_CACHE = (compiled, zeros, shardings)
    return _COMPILED_CACHE


def _pack_emb16(embeddings: np.ndarray) -> np.ndarray:
    emb16 = np.zeros((NUM_LEVELS, NE, FEAT), np.float16)
    for l in range(NUM_LEVELS):
        c = int(COUNTS[l])
        emb16[l, :c] = embeddings[int(OFFSETS[l]):int(OFFSETS[l]) + c].astype(np.float16)
    return emb16.reshape(NUM_LEVELS, NE * FEAT)


def _decode(q: np.ndarray, s: np.ndarray) -> np.ndarray:
    scl = s.astype(np.float32) * np.float32(1.0 / 127.0)
    return np.multiply(q, scl[:, None], dtype=np.float32)


def _fast_call(coords: np.ndarray, emb16: np.ndarray) -> np.ndarray:
    compiled, zeros, sh = _get_compiled()
    # emb: 1MB to dev0, then D2D broadcast (direct replicated put ships
    # 8 host copies over the ~45MB/s tunnel). All puts enqueue async;
    # chunk i+1's coords upload and exec overlap chunk i's exec and fetch.
    de0 = jax.device_put(emb16, sh["dev0"])
    drep = jax.device_put(de0, sh["rep"])
    results = []
    for h in range(N_SPLIT):
        # quantize per chunk so chunk 0's upload starts immediately
        cq = np.clip(coords[h * NH:(h + 1) * NH] * np.float32(65536.0),
                     0, 65535).astype(np.uint16)
        dc = jax.device_put(cq, sh["core"])
        results.append(compiled(dc, drep, *zeros))

    # queue all output transfers, then decode shards as they land
    for q, s in results:
        s.copy_to_host_async()
        for x in q.addressable_shards:
            x.data.copy_to_host_async()

    out = np.empty((N_POINTS, 32), np.float32)
    with ThreadPoolExecutor(2) as ex:
        futs = []
        for h, (q, s) in enumerate(results):
            scl = np.asarray(s).astype(np.float32) * np.float32(1.0 / 127.0)
            shards = sorted(q.addressable_shards,
                            key=lambda x: x.index[0].start or 0)
            for x in shards:
                qa = np.asarray(x.data)  # blocks until this shard lands
                st = (x.index[0].start or 0) + h * NH
                futs.append(ex.submit(
                    np.multiply, qa, scl[st - h * NH:st - h * NH + qa.shape[0], None],
                    out=out[st:st + qa.shape[0]]))
        for f in futs:
            f.result()
    return out


def kernel(coords: np.ndarray, embeddings: np.ndarray) -> np.ndarray:
    coords = np.asarray(coords, dtype=np.float32)
    embeddings = np.asarray(embeddings, dtype=np.float32)
    emb16 = _pack_emb16(embeddings)

    try:
        # u16 fixed-point coords (halves the upload; trilinear interp is
        # continuous in coords, so the 2^-16 snap costs ~2e-3 rel err)
        return _fast_call(coords, emb16)
    except Exception:
        nc = _get_nc()
        cq = np.clip(coords * np.float32(65536.0), 0, 65535).astype(np.uint16)
        outs = []
        for h in range(N_SPLIT):
            ch = cq[h * NH:(h + 1) * NH]
            in_maps = [
                {"coords": ch[c * NC_N:(c + 1) * NC_N], "emb16": emb16}
                for c in range(N_CORES)
            ]
            res = run_bass_kernel_spmd(nc, in_maps,
                                       core_ids=list(range(N_CORES)))
            q = np.concatenate([r["outq"] for r in res.results], axis=0)
            s = np.concatenate([r["outs"] for r in res.results], axis=0)
            outs.append(_decode(q, s))
        return np.concatenate(outs, axis=0)


# revision 38
# speedup vs baseline: 1.2046x; 1.2046x over previous
"""Instant-NGP hash-encoding forward on 8 TRN2 NeuronCores.

Data-parallel over points (8 cores). Per core:
  - fp16 per-level tables (padded to 16384 entries) broadcast-DMA'd into SBUF
    across all 128 partitions (64KB/partition), one level at a time.
  - Gather via GPSIMD ap_gather (8 Q7 cores/NC work in parallel on their own
    16-partition groups).
  - DVE computes cell coords (floor with round-to-nearest fix), spatial-hash
    indices (int32 mul/and/xor; mod 2^14 == low-14-bit arithmetic), trilinear
    weights, and the 8-corner weighted reduction.
  - Per-level results go to a DRAM scratch (level-major); a final pass
    interleaves them into the [N, 32] output (fp16 to halve the fetch).

Host path: one cached AOT-compiled jax executable (fresh jit per call would
retrace + recompile); output zero-buffers are created on device instead of
being shipped; output returns as fp16 and is upcast host-side. All of this
matters because the axon tunnel runs at ~45MB/s each way.
"""
from concurrent.futures import ThreadPoolExecutor

import numpy as np

import jax
import jax.numpy as jnp
from jax.sharding import (Mesh, PartitionSpec, NamedSharding,
                          SingleDeviceSharding)
from jax.experimental.shard_map import shard_map

import concourse.bass as bass
import concourse.mybir as mybir
from concourse import bacc, bass2jax
from concourse.tile import TileContext
from concourse.bass_utils import run_bass_kernel_spmd

F32 = mybir.dt.float32
F16 = mybir.dt.float16
I32 = mybir.dt.int32
I16 = mybir.dt.int16
I8 = mybir.dt.int8
U16 = mybir.dt.uint16
AL = mybir.AluOpType
AX = mybir.AxisListType

NUM_LEVELS = 16
TABLE_SIZE = 2 ** 14
MIN_RES, MAX_RES = 16, 512
FEAT = 2
N_POINTS = 1 << 20
N_CORES = 8
PI1, PI2 = 2654435761, 805459861
P1L = PI1 & (TABLE_SIZE - 1)
P2L = PI2 & (TABLE_SIZE - 1)

_b = np.exp((np.log(MAX_RES) - np.log(MIN_RES)) / (NUM_LEVELS - 1))
RES = np.floor(MIN_RES * _b ** np.arange(NUM_LEVELS)).astype(np.int64)
COUNTS = np.minimum((RES + 1) ** 3, TABLE_SIZE)
OFFSETS = np.concatenate([[0], np.cumsum(COUNTS)])
DENSE = [int(COUNTS[l]) == int((RES[l] + 1) ** 3) for l in range(NUM_LEVELS)]

# Sequential chunk-calls: exec(i+1)/upload(i+1) overlap fetch(i). Uniform
# quarters beat a small-first/big-tail schedule in a back-to-back A/B
# (1007ms vs 1026ms best, and much tighter variance).
SCHEDULE = [N_POINTS // 4] * 4
P = 128
T = 64                           # points per partition per tile
NI = 16 * T * 8                  # ap_gather num_idxs per 16-partition group
NE = TABLE_SIZE

# f32 blob slots (units of T elements)
S_SX, S_XF, S_GT = 0, 1, 2
S_FL = 3   # 3 slots
S_FR = 6   # 3 slots
S_W0 = 9   # 3 slots
S_WXY = 12  # 4 slots
S_PROD = 16  # 16 slots
NBF = 32
# i32 blob slots
S_XI = 0
S_FI = 1   # 3 slots
S_HX1 = 4
S_HY0, S_HY1, S_HZ0, S_HZ1 = 5, 6, 7, 8
S_TMP = 9
S_HXY = 10  # 4 slots
S_IDX = 14  # 8 slots
NBI = 22


def _ap(tile_ap, part_off, part_step, part_cnt, elem_off, dims):
    pitch = tile_ap.ap[0][0]
    return bass.AP(
        tile_ap.tensor,
        tile_ap.offset + part_off * pitch + elem_off,
        [[part_step * pitch, part_cnt]] + dims,
    )


def _build_nc(nc_n):
    PPP = nc_n // P              # points per partition
    NT = PPP // T                # tiles per core
    nc = bacc.Bacc("TRN2", target_bir_lowering=False, debug=False)
    coords = nc.dram_tensor("coords", [nc_n, 3], U16, kind="ExternalInput")
    emb16 = nc.dram_tensor("emb16", [NUM_LEVELS, NE * FEAT], F16, kind="ExternalInput")
    # int8 quantized output + per-point fp16 scale: out = q * s / 127.
    outq = nc.dram_tensor("outq", [nc_n, 32], I8, kind="ExternalOutput")
    outs = nc.dram_tensor("outs", [nc_n], F16, kind="ExternalOutput")

    coords_v = coords[:, :].rearrange("(p q) c -> p (q c)", p=P)
    outq_v = outq[:, :].rearrange("(p q) g -> p (q g)", p=P)
    outs_v = outs[:].rearrange("(p q) -> p q", p=P)

    with TileContext(nc) as tc:
        with tc.tile_pool(name="dram", bufs=1, space="DRAM") as dpool, \
             tc.tile_pool(name="tab", bufs=1) as tabp, \
             tc.tile_pool(name="coord", bufs=1) as cpool, \
             tc.tile_pool(name="gat", bufs=2) as gpool, \
             tc.tile_pool(name="blob", bufs=2) as bp, \
             tc.tile_pool(name="idxw", bufs=2) as xp, \
             tc.tile_pool(name="io", bufs=2) as iop:

            scratch = dpool.tile([NUM_LEVELS * NT * P, T * FEAT], F32)

            ct = cpool.tile([P, PPP * 3], U16)
            nc.sync.dma_start(out=ct[:], in_=coords_v)

            for l in range(NUM_LEVELS):
                R = int(RES[l])
                tab = tabp.tile([P, NE * FEAT], F16, tag="tab")
                nc.sync.dma_start(
                    out=tab[:], in_=emb16[l:l + 1, :].to_broadcast([P, NE * FEAT])
                )
                for ti in range(NT):
                    co = ti * T * 3
                    cap = ct[:]
                    cviews = [
                        bass.AP(cap.tensor, cap.offset + co + a, [cap.ap[0], [3, T]])
                        for a in range(3)
                    ]

                    bf = bp.tile([P, NBF * T], F32, tag="bf")
                    bi = bp.tile([P, NBI * T], I32, tag="bi")

                    def fv(slot, dims=None, off=0):
                        return _ap(bf[:], 0, 1, P, slot * T + off, dims or [[1, T]])

                    def iv(slot, dims=None, off=0):
                        return _ap(bi[:], 0, 1, P, slot * T + off, dims or [[1, T]])

                    # floor + frac per axis (coords are u16 fixed-point / 2^16)
                    for a in range(3):
                        nc.vector.tensor_copy(out=fv(S_GT), in_=cviews[a])
                        nc.vector.tensor_scalar(out=fv(S_SX), in0=fv(S_GT),
                                                scalar1=float(R) / 65536.0,
                                                scalar2=None, op0=AL.mult)
                        nc.vector.tensor_copy(out=iv(S_XI), in_=fv(S_SX))
                        nc.vector.tensor_copy(out=fv(S_XF), in_=iv(S_XI))
                        nc.vector.tensor_tensor(out=fv(S_GT), in0=fv(S_XF), in1=fv(S_SX),
                                                op=AL.is_gt)
                        nc.vector.tensor_tensor(out=fv(S_FL + a), in0=fv(S_XF),
                                                in1=fv(S_GT), op=AL.subtract)
                        nc.vector.tensor_tensor(out=fv(S_FR + a), in0=fv(S_SX),
                                                in1=fv(S_FL + a), op=AL.subtract)
                        nc.vector.tensor_copy(out=iv(S_FI + a), in_=fv(S_FL + a))

                    if DENSE[l]:
                        Rp = R + 1
                        nc.vector.tensor_scalar(out=iv(S_HX1), in0=iv(S_FI + 1),
                                                scalar1=Rp, scalar2=None, op0=AL.mult)
                        nc.vector.tensor_tensor(out=iv(S_HY0), in0=iv(S_HX1),
                                                in1=iv(S_FI + 0), op=AL.add)
                        nc.vector.tensor_scalar(out=iv(S_HY1), in0=iv(S_FI + 2),
                                                scalar1=Rp * Rp, scalar2=None, op0=AL.mult)
                        nc.vector.tensor_tensor(out=iv(S_HZ0), in0=iv(S_HY0),
                                                in1=iv(S_HY1), op=AL.add)
                        for c in range(8):
                            i, j, k = (c >> 2) & 1, (c >> 1) & 1, c & 1
                            doff = i + Rp * j + Rp * Rp * k
                            ov = iv(S_IDX, [[8, T]], off=c)
                            nc.vector.tensor_scalar(out=ov, in0=iv(S_HZ0), scalar1=doff,
                                                    scalar2=None, op0=AL.add)
                    else:
                        nc.vector.tensor_scalar(out=iv(S_HX1), in0=iv(S_FI + 0),
                                                scalar1=1, scalar2=None, op0=AL.add)
                        for ax, pl, s0, s1 in ((1, P1L, S_HY0, S_HY1),
                                               (2, P2L, S_HZ0, S_HZ1)):
                            nc.vector.tensor_scalar(out=iv(S_TMP), in0=iv(S_FI + ax),
                                                    scalar1=pl, scalar2=None, op0=AL.mult)
                            nc.vector.tensor_scalar(out=iv(s0), in0=iv(S_TMP),
                                                    scalar1=NE - 1, scalar2=None,
                                                    op0=AL.bitwise_and)
                            nc.vector.tensor_scalar(out=iv(S_TMP), in0=iv(s0),
                                                    scalar1=pl, scalar2=None, op0=AL.add)
                            nc.vector.tensor_scalar(out=iv(s1), in0=iv(S_TMP),
                                                    scalar1=NE - 1, scalar2=None,
                                                    op0=AL.bitwise_and)
                        for i in range(2):
                            hxs = iv(S_FI + 0) if i == 0 else iv(S_HX1)
                            for j in range(2):
                                ov = iv(S_HXY, [[4, T]], off=i * 2 + j)
                                nc.vector.tensor_tensor(out=ov, in0=hxs,
                                                        in1=iv(S_HY0 if j == 0 else S_HY1),
                                                        op=AL.bitwise_xor)
                        for c in range(8):
                            i, j, k = (c >> 2) & 1, (c >> 1) & 1, c & 1
                            inv = iv(S_HXY, [[4, T]], off=i * 2 + j)
                            ov = iv(S_IDX, [[8, T]], off=c)
                            nc.vector.tensor_tensor(out=ov, in0=inv,
                                                    in1=iv(S_HZ0 if k == 0 else S_HZ1),
                                                    op=AL.bitwise_xor)

                    idx16 = xp.tile([P, T * 8], I16, tag="idx16")
                    nc.vector.tensor_copy(out=idx16[:],
                                          in_=iv(S_IDX, [[1, 8 * T]]))

                    # weights
                    for a in range(3):
                        nc.vector.tensor_scalar(out=fv(S_W0 + a), in0=fv(S_FR + a),
                                                scalar1=-1.0, scalar2=1.0,
                                                op0=AL.mult, op1=AL.add)
                    for i in range(2):
                        for j in range(2):
                            ov = fv(S_WXY, [[4, T]], off=i * 2 + j)
                            nc.vector.tensor_tensor(
                                out=ov, in0=fv(S_W0 + 0 if i == 0 else S_FR + 0),
                                in1=fv(S_W0 + 1 if j == 0 else S_FR + 1), op=AL.mult)
                    wt = xp.tile([P, T * 8], F32, tag="wt")
                    for c in range(8):
                        i, j, k = (c >> 2) & 1, (c >> 1) & 1, c & 1
                        inv = fv(S_WXY, [[4, T]], off=i * 2 + j)
                        ov = _ap(wt[:], 0, 1, P, c, [[8, T]])
                        nc.vector.tensor_tensor(out=ov, in0=inv,
                                                in1=fv(S_W0 + 2 if k == 0 else S_FR + 2),
                                                op=AL.mult)

                    gat = gpool.tile([P, NI * FEAT], F16, tag="gat")
                    nc.gpsimd.ap_gather(
                        out_ap=gat[:], in_ap=tab[:], idxs_ap=idx16[:],
                        channels=P, num_elems=NE, d=FEAT, num_idxs=NI,
                    )

                    # de-interleave: partition 16g+j's results live at slots
                    # s*16+j (replicated across the group); 16 partition-subset
                    # DMAs bring each partition its own (t,c,f)-ordered copy.
                    gx = xp.tile([P, T * 16], F16, tag="gx")
                    for j in range(16):
                        src = _ap(gat[:], j, 16, 8, j * 2, [[32, 8 * T], [1, 2]])
                        dst = _ap(gx[:], j, 16, 8, 0, [[1, 16 * T]])
                        nc.sync.dma_start(out=dst, in_=src)

                    res = iop.tile([P, T * FEAT], F32, tag="res")
                    gv = gx[:].rearrange("p (t c f) -> p t f c", c=8, f=2)
                    wv = _ap(wt[:], 0, 1, P, 0, [[8, T], [0, 2], [1, 8]])
                    pv = _ap(bf[:], 0, 1, P, S_PROD * T, [[16, T], [8, 2], [1, 8]])
                    nc.vector.tensor_tensor(out=pv, in0=gv, in1=wv, op=AL.mult)
                    pv2 = _ap(bf[:], 0, 1, P, S_PROD * T, [[16, T], [8, 2], [1, 8]])
                    rv = res[:].rearrange("p (t f) -> p t f", f=2)
                    nc.vector.tensor_reduce(out=rv, in_=pv2, axis=AX.X, op=AL.add)

                    row = (l * NT + ti) * P
                    nc.sync.dma_start(out=scratch[row:row + P, :], in_=res[:])

            for ti in range(NT):
                asm = iop.tile([P, T * 32], F16, tag="asm")
                for l in range(NUM_LEVELS):
                    slab = iop.tile([P, T * FEAT], F32, tag="slab")
                    row = (l * NT + ti) * P
                    nc.sync.dma_start(out=slab[:], in_=scratch[row:row + P, :])
                    av = asm[:].rearrange("p (t g) -> p t g", g=32)[:, :, 2 * l:2 * l + 2]
                    sv = slab[:].rearrange("p (t f) -> p t f", f=FEAT)
                    nc.vector.tensor_copy(out=av, in_=sv)

                # int8 quantize with per-point scale: s16 = f16(max|x|),
                # q = round(x * 127 / s16); host decodes q * (s16/127).
                babs = iop.tile([P, T * 32], F16, tag="babs")
                s16p = iop.tile([P, T], F16, tag="s16p")
                s16 = iop.tile([P, T], F16, tag="s16")
                rs = iop.tile([P, T], F32, tag="rs")
                rs2 = iop.tile([P, T], F32, tag="rs2")
                rs3 = iop.tile([P, T], F32, tag="rs3")
                q8 = iop.tile([P, T * 32], I8, tag="q8")
                av3 = asm[:].rearrange("p (t g) -> p t g", g=32)
                # |x| = clear the f16 sign bit
                nc.vector.tensor_scalar(out=babs[:].bitcast(I16),
                                        in0=asm[:].bitcast(I16),
                                        scalar1=0x7FFF, scalar2=None,
                                        op0=AL.bitwise_and)
                ab3 = babs[:].rearrange("p (t g) -> p t g", g=32)
                nc.vector.tensor_reduce(out=s16p[:], in_=ab3, axis=AX.X,
                                        op=AL.max)
                nc.vector.tensor_scalar(out=s16[:], in0=s16p[:], scalar1=1e-6,
                                        scalar2=None, op0=AL.max)
                nc.vector.tensor_copy(out=rs[:], in_=s16[:])
                nc.vector.reciprocal(out=rs2[:], in_=rs[:])
                nc.vector.tensor_scalar(out=rs3[:], in0=rs2[:], scalar1=127.0,
                                        scalar2=None, op0=AL.mult)
                q3 = q8[:].rearrange("p (t g) -> p t g", g=32)
                scb = _ap(rs3[:], 0, 1, P, 0, [[1, T], [0, 32]])
                nc.vector.tensor_tensor(out=q3, in0=av3, in1=scb, op=AL.mult)
                nc.sync.dma_start(
                    out=bass.AP(outq_v.tensor, outq_v.offset + ti * T * 32,
                                [outq_v.ap[0], [1, T * 32]]),
                    in_=q8[:],
                )
                nc.sync.dma_start(
                    out=bass.AP(outs_v.tensor, outs_v.offset + ti * T,
                                [outs_v.ap[0], [1, T]]),
                    in_=s16[:],
                )
    nc.compile()
    return nc


_NC_CACHE = {}
_COMPILED_CACHE = {}


def _get_nc(nc_n):
    if nc_n not in _NC_CACHE:
        _NC_CACHE[nc_n] = _build_nc(nc_n)
    return _NC_CACHE[nc_n]


def _get_compiled(n_call):
    """One AOT-compiled 8-device executable per chunk size, cached for the
    process.

    Mirrors run_bass_via_pjrt's lowering but (a) is compiled once instead of
    per call, (b) materializes the ExternalOutput zero-buffers on device
    (saves a 64MB host->device ship per call), (c) uses the replicated
    sharding for the embedding table instead of 8 concatenated copies.
    """
    if n_call not in _COMPILED_CACHE:
        nc = _get_nc(n_call // N_CORES)
        bass2jax.install_neuronx_cc_hook()
        assert nc.dbg_addr is None
        partition_name = (
            nc.partition_id_tensor.name if nc.partition_id_tensor else None
        )

        in_names, out_names, out_avals = [], [], []
        for alloc in nc.m.functions[0].allocations:
            if not isinstance(alloc, mybir.MemoryLocationSet):
                continue
            name = alloc.memorylocations[0].name
            if alloc.kind == "ExternalInput":
                if name != partition_name:
                    in_names.append(name)
            elif alloc.kind == "ExternalOutput":
                out_names.append(name)
                out_avals.append(jax.core.ShapedArray(
                    tuple(alloc.tensor_shape), mybir.dt.np(alloc.dtype)))
        assert in_names == ["coords", "emb16"], in_names
        assert out_names == ["outq", "outs"], out_names
        bind_in_names = tuple(in_names) + tuple(out_names)
        if partition_name is not None:
            bind_in_names = bind_in_names + (partition_name,)

        devices = jax.devices()[:N_CORES]
        mesh = Mesh(np.asarray(devices), ("core",))

        def _body(coords_l, emb_l, zq_l, zs_l):
            operands = [coords_l, emb_l, zq_l, zs_l]
            if partition_name is not None:
                operands.append(bass2jax.partition_id_tensor())
            outs = bass2jax._bass_exec_p.bind(
                *operands,
                out_avals=tuple(out_avals),
                in_names=bind_in_names,
                out_names=tuple(out_names),
                lowering_input_output_aliases=(),
                sim_require_finite=True,
                sim_require_nnan=True,
                nc=nc,
            )
            return tuple(outs)

        fn = shard_map(
            _body, mesh=mesh,
            in_specs=(PartitionSpec("core"), PartitionSpec(),
                      PartitionSpec("core"), PartitionSpec("core")),
            out_specs=(PartitionSpec("core"), PartitionSpec("core")),
            check_rep=False,
        )
        sh_core = jax.sharding.NamedSharding(mesh, PartitionSpec("core"))
        # The output "zero buffer" parameters run_bass_via_pjrt ships from the
        # host every call; our kernel writes every output element, so
        # device-resident dummies (never donated, so they survive calls) work.
        zeros = jax.jit(
            lambda: (jnp.zeros((n_call, 32), jnp.int8),
                     jnp.zeros((n_call,), jnp.float16)),
            out_shardings=(sh_core, sh_core),
        )()
        jax.block_until_ready(zeros)
        cshape = jax.ShapeDtypeStruct((n_call, 3), np.uint16)
        eshape = jax.ShapeDtypeStruct((NUM_LEVELS, NE * FEAT), np.float16)
        zqshape = jax.ShapeDtypeStruct((n_call, 32), np.int8)
        zsshape = jax.ShapeDtypeStruct((n_call,), np.float16)
        compiled = bass2jax.fast_dispatch_compile(
            lambda: jax.jit(fn).lower(cshape, eshape, zqshape, zsshape).compile()
        )
        shardings = {
            "core": sh_core,
            "rep": jax.sharding.NamedSharding(mesh, PartitionSpec()),
            "dev0": SingleDeviceSharding(devices[0]),
        }
        _COMPILED_CACHE[n_call] = (compiled, zeros, shardings)
    return _COMPILED_CACHE[n_call]


def _pack_emb16(embeddings: np.ndarray) -> np.ndarray:
    emb16 = np.zeros((NUM_LEVELS, NE, FEAT), np.float16)
    for l in range(NUM_LEVELS):
        c = int(COUNTS[l])
        emb16[l, :c] = embeddings[int(OFFSETS[l]):int(OFFSETS[l]) + c].astype(np.float16)
    return emb16.reshape(NUM_LEVELS, NE * FEAT)


def _decode(q: np.ndarray, s: np.ndarray) -> np.ndarray:
    scl = s.astype(np.float32) * np.float32(1.0 / 127.0)
    return np.multiply(q, scl[:, None], dtype=np.float32)


def _fast_call(coords: np.ndarray, emb16: np.ndarray) -> np.ndarray:
    states = [_get_compiled(n) for n in SCHEDULE]
    sh = states[0][2]
    # emb: 1MB to dev0, then D2D broadcast (direct replicated put ships
    # 8 host copies over the ~45MB/s tunnel). All puts enqueue async;
    # chunk i+1's coords upload and exec overlap chunk i's exec and fetch.
    de0 = jax.device_put(emb16, sh["dev0"])
    drep = jax.device_put(de0, sh["rep"])
    results = []
    base = 0
    for n, (compiled, zeros, shn) in zip(SCHEDULE, states):
        # quantize per chunk so chunk 0's upload starts immediately
        cq = np.clip(coords[base:base + n] * np.float32(65536.0),
                     0, 65535).astype(np.uint16)
        dc = jax.device_put(cq, shn["core"])
        res = compiled(dc, drep, *zeros)
        q, s = res
        s.copy_to_host_async()
        for x in q.addressable_shards:
            x.data.copy_to_host_async()
        results.append((base, res))
        base += n

    out = np.empty((N_POINTS, 32), np.float32)
    with ThreadPoolExecutor(2) as ex:
        futs = []
        for base, (q, s) in results:
            scl = np.asarray(s).astype(np.float32) * np.float32(1.0 / 127.0)
            shards = sorted(q.addressable_shards,
                            key=lambda x: x.index[0].start or 0)
            for x in shards:
                qa = np.asarray(x.data)  # blocks until this shard lands
                st = x.index[0].start or 0
                futs.append(ex.submit(
                    np.multiply, qa, scl[st:st + qa.shape[0], None],
                    out=out[base + st:base + st + qa.shape[0]]))
        for f in futs:
            f.result()
    return out


def kernel(coords: np.ndarray, embeddings: np.ndarray) -> np.ndarray:
    coords = np.asarray(coords, dtype=np.float32)
    embeddings = np.asarray(embeddings, dtype=np.float32)
    emb16 = _pack_emb16(embeddings)

    try:
        # u16 fixed-point coords (halves the upload; trilinear interp is
        # continuous in coords, so the 2^-16 snap costs ~2e-3 rel err)
        return _fast_call(coords, emb16)
    except Exception:
        cq = np.clip(coords * np.float32(65536.0), 0, 65535).astype(np.uint16)
        outs = []
        base = 0
        for n in SCHEDULE:
            nc_n = n // N_CORES
            nc = _get_nc(nc_n)
            ch = cq[base:base + n]
            in_maps = [
                {"coords": ch[c * nc_n:(c + 1) * nc_n], "emb16": emb16}
                for c in range(N_CORES)
            ]
            res = run_bass_kernel_spmd(nc, in_maps,
                                       core_ids=list(range(N_CORES)))
            q = np.concatenate([r["outq"] for r in res.results], axis=0)
            s = np.concatenate([r["outs"] for r in res.results], axis=0)
            outs.append(_decode(q, s))
            base += n
        return np.concatenate(outs, axis=0)
